# revision 12
# baseline (speedup 1.0000x reference)
"""Trainium2 Bass kernel for nn_CDFVarianceLoss.

Math (per sample b, per tensor z in {pred[b], target[b]}, N = 65536):
    z' = (z - min z) / (max z - min z + 1e-6)
    h_j = sum_n exp(-(z'_n - c_j)^2 / (2*sigma^2)) + 1e-6,  c_j = j/63, j < 64
    cdf = cumsum(h / sum_j h)
    loss = mean_{b,j} (cdf_pred[b,j] - cdf_target[b,j])^2

Key identity: the cumulative kernel sums S_j = sum_n V_j(z'_n) with
V_j(z) = sum_{k<=j} exp(-alpha (z-c_k)^2) are smooth sigmoid-like functions
of z.  Each V_j is approximated (offline least squares) by a linear
combination of M=2 shifted erfs of common width plus a constant:
    V_j(z) ~= R[j,0] erf((z-g_0)/s) + R[j,1] erf((z-g_1)/s) + R[j,2]
so the device only computes the two basis sums D_k = sum_n erf((z'_n-g_k)/s)
per array and the host applies R in float64 plus the exact
eps/normalize/cumsum/mse tail.

Device layout: the 4 arrays per core (pred/target x 2 samples) are stacked
along the PARTITION axis -- array a owns partitions [32a, 32a+32), each
partition holding 2048 contiguous elements.  This makes the per-array
normalization constants per-partition vectors, which fuse directly into the
erf activation via its per-partition scale/bias operands: no separate
normalize pass.  The activation's accumulate output (accum_out) produces the
per-partition free-dim sums, so no vector-engine reduction chase either.

Per-core pipeline:
  - 6 chunked loads (free-chunks 512/512/1024 x {pred, target}, each
    spanning 2 samples = 64 partitions) over the three DMA-capable queues;
    DVE min/max reduces run per chunk as the data lands
  - cross-partition (per 32-partition group) min/max via a 5-round
    stream_shuffle XOR butterfly + tensor_tensor max, all on DVE
  - 5 small DVE ops turn (gmax, -gmin) into ACT scale/bias columns
  - 2 Erf activation passes [128, 2048] with per-partition scale/bias and
    accum_out -> red[128, 2]
  - red goes to DRAM; the host sums the 32-partition groups, applies R and
    computes the exact eps/normalize/cumsum/mse tail in float64
"""

import math

import numpy as np

B = 16
N = 65536
BINS = 64
SIGMA = 0.05
EPS = 1e-6
ALPHA = 0.5 / SIGMA**2  # 200.0
NCORES = 8
SPC = B // NCORES  # samples per core
NARR = 2 * SPC  # arrays per core: (pred, target) x samples
P = 128
GRP = P // NARR  # partitions per array
FT = N // GRP  # free dim per partition: 2048

# erf basis (designed offline; see module docstring)
M = 2
G0 = 0.397217865
G1 = 0.599198046
S_W = 0.136065682
CHUNKS = [0, 640, 1280, 2048]

_CACHE = {}


def _fit_R():
    """Least-squares fit of V_j(z) in the erf basis + constant (fp64)."""
    nz = 40001
    zg = np.linspace(0.0, 1.0, nz)
    c = np.linspace(0.0, 1.0, BINS)
    K = np.exp(-ALPHA * (zg[None, :] - c[:, None]) ** 2)
    Vt = np.cumsum(K, axis=0)  # [64, nz]
    erf_v = np.vectorize(math.erf)
    Fb = np.concatenate(
        [
            erf_v((zg[None, :] - np.array([[G0], [G1]])) / S_W),
            np.ones((1, nz)),
        ],
        axis=0,
    )  # [M+1, nz]
    w = np.ones(nz)
    w[zg < 0.02] = 3.0
    w[zg > 0.98] = 3.0
    Aw = Fb * w[None, :]
    Gm = Aw @ Fb.T
    Rhs = (Vt * w[None, :]) @ Fb.T
    return np.linalg.solve(Gm + 1e-10 * np.eye(M + 1), Rhs.T).T  # [64, M+1]


def _build_nc():
    import concourse.bass as bass  # noqa: F401
    import concourse.bacc as bacc
    import concourse.tile as tile
    from concourse import mybir
    from contextlib import ExitStack

    f32 = mybir.dt.float32
    f16 = mybir.dt.float16
    AX = mybir.AxisListType
    OP = mybir.AluOpType
    ACTF = mybir.ActivationFunctionType

    nc = bacc.Bacc()
    pred_d = nc.declare_dram_parameter("pred", [SPC, N], f32, isOutput=False)
    targ_d = nc.declare_dram_parameter("target", [SPC, N], f32, isOutput=False)
    out_d = nc.declare_dram_parameter("dsums", [P, M], f32, isOutput=True)

    with tile.TileContext(nc) as tc, ExitStack() as ctx:
        nat = ctx.enter_context(tc.tile_pool(name="nat", bufs=1))
        eo_pool = ctx.enter_context(tc.tile_pool(name="eo", bufs=2))
        small = ctx.enter_context(tc.tile_pool(name="small", bufs=1))

        # partition-stacked input: array a -> partitions [32a, 32a+32);
        # each dma spans one tensor (2 samples = 64 partitions) x free-chunk
        zn = nat.tile([P, FT], f32, name="zn")
        t2d = [
            pred_d.rearrange("s (p f) -> (s p) f", p=GRP),
            targ_d.rearrange("s (p f) -> (s p) f", p=GRP),
        ]
        # chunk 0 rides the two HWDGE queues (sync/scalar start ~0.65us
        # earlier than the gpsimd SWDGE path), so the DVE pipeline starts
        # as soon as possible; later chunks absorb the gpsimd latency
        qs = [nc.sync, nc.scalar, nc.gpsimd, nc.sync, nc.scalar, nc.gpsimd]
        qi = 0
        NCH = len(CHUNKS) - 1
        for ci in range(NCH):
            c0, c1 = CHUNKS[ci], CHUNKS[ci + 1]
            for t in range(2):
                qs[qi].dma_start(
                    out=zn[t * 2 * GRP : (t + 1) * 2 * GRP, c0:c1],
                    in_=t2d[t][:, c0:c1],
                )
                qi += 1

        # per-partition min/max, chunk-pipelined with the DMA
        mmp = small.tile([P, 2 * NCH], f32, tag="mmp")  # max cols | negmin cols
        for ci in range(NCH):
            c0, c1 = CHUNKS[ci], CHUNKS[ci + 1]
            nc.vector.tensor_reduce(
                out=mmp[:, ci : ci + 1], in_=zn[:, c0:c1], axis=AX.X, op=OP.max
            )
            nc.vector.tensor_reduce(
                out=mmp[:, NCH + ci : NCH + ci + 1], in_=zn[:, c0:c1], axis=AX.X,
                op=OP.min, negate=True,
            )
        mm = small.tile([P, 2], f32, tag="mm")  # (max, -min) per partition
        nc.vector.tensor_reduce(
            out=mm, in_=mmp.rearrange("p (t c) -> p t c", c=NCH), axis=AX.X,
            op=OP.max,
        )

        # 32-partition-group all-reduce max via stream_shuffle XOR butterfly
        cur = mm
        for step in (16, 8, 4, 2, 1):
            mask = [i ^ step for i in range(32)]
            sh = small.tile([P, 2], f32, tag=f"sh{step}")
            nc.vector.stream_shuffle(sh, cur, mask)
            nxt = small.tile([P, 2], f32, tag=f"bt{step}")
            nc.vector.tensor_tensor(out=nxt, in0=cur, in1=sh, op=OP.max)
            cur = nxt
        # cur[p] = (gmax, -gmin) of partition p's array, on every partition

        # scale/bias columns: nb = [inv/S | -gmin*inv/S - G0/S | ... - G1/S],
        # inv = 1/(gmax-gmin); bias_k fused as (-gmin)*(inv/S) + (-Gk/S)
        nb = small.tile([P, 1 + M], f32, tag="nb")
        rngs = small.tile([P, 1], f32, tag="rngs")
        nc.vector.tensor_scalar(
            rngs, cur[:, 0:1], cur[:, 1:2], float(S_W), OP.add, OP.mult
        )
        nc.vector.reciprocal(nb[:, 0:1], rngs)
        nc.vector.tensor_scalar(
            nb[:, 1:2], cur[:, 1:2], nb[:, 0:1], float(-G0 / S_W), OP.mult, OP.add
        )
        nc.vector.tensor_scalar(
            nb[:, 2:3], cur[:, 1:2], nb[:, 0:1], float(-G1 / S_W), OP.mult, OP.add
        )

        # erf spine: arg = z*(inv/S) + bias_k = (z' - g_k)/S; accum_out gives
        # the per-partition free-dim sums directly
        red = small.tile([P, M], f32, tag="red")
        for k in range(M):
            eo = eo_pool.tile([P, FT], f16, tag="eo")
            nc.scalar.activation(
                out=eo,
                in_=zn,
                func=ACTF.Erf,
                bias=nb[:, 1 + k : 2 + k],
                scale=nb[:, 0:1],
                accum_out=red[:, k : k + 1],
            )

        # issue from the scalar queue: the accum-read that produces red runs
        # on Scalar, so no cross-engine semaphore hop before the out DMA
        nc.scalar.dma_start(out=out_d[:, :], in_=red)

    nc.compile()
    return nc


def kernel(pred: np.ndarray, target: np.ndarray) -> np.ndarray:
    from concourse.bass_utils import run_bass_kernel_spmd

    if "nc" not in _CACHE:
        _CACHE["nc"] = _build_nc()
        _CACHE["R"] = _fit_R()
    nc = _CACHE["nc"]
    R = _CACHE["R"]

    pred = np.ascontiguousarray(np.asarray(pred, np.float32).reshape(B, N))
    target = np.ascontiguousarray(np.asarray(target, np.float32).reshape(B, N))
    in_maps = [
        {
            "pred": pred[i * SPC : (i + 1) * SPC],
            "target": target[i * SPC : (i + 1) * SPC],
        }
        for i in range(NCORES)
    ]
    res = run_bass_kernel_spmd(nc, in_maps, list(range(NCORES)))

    # device dsums [128, M]; array a = partition rows [32a, 32a+32) with
    # a in (pred_s0, pred_s1, targ_s0, targ_s1); host sums the groups
    Dp = np.zeros((B, M + 1))
    Dt = np.zeros((B, M + 1))
    Dp[:, M] = N
    Dt[:, M] = N
    for core in range(NCORES):
        raw = np.asarray(res.results[core]["dsums"], np.float64)  # [128, M]
        for s in range(SPC):
            b = core * SPC + s
            Dp[b, :M] = raw[s * GRP : (s + 1) * GRP, :].sum(axis=0)
            Dt[b, :M] = raw[(SPC + s) * GRP : (SPC + s + 1) * GRP, :].sum(axis=0)

    Sx = Dp @ R.T  # [B, 64] cumulative kernel sums
    Sy = Dt @ R.T
    js = np.arange(1, BINS + 1, dtype=np.float64)
    cdf_x = (Sx + js[None, :] * EPS) / (Sx[:, -1:] + BINS * EPS)
    cdf_y = (Sy + js[None, :] * EPS) / (Sy[:, -1:] + BINS * EPS)
    return np.float32(np.mean((cdf_x - cdf_y) ** 2))


# revision 15
# speedup vs baseline: 1.0862x; 1.0862x over previous
"""Trainium2 Bass kernel for nn_CDFVarianceLoss.

Math (per sample b, per tensor z in {pred[b], target[b]}, N = 65536):
    z' = (z - min z) / (max z - min z + 1e-6)
    h_j = sum_n exp(-(z'_n - c_j)^2 / (2*sigma^2)) + 1e-6,  c_j = j/63, j < 64
    cdf = cumsum(h / sum_j h)
    loss = mean_{b,j} (cdf_pred[b,j] - cdf_target[b,j])^2

Key identity: the cumulative kernel sums S_j = sum_n V_j(z'_n) with
V_j(z) = sum_{k<=j} exp(-alpha (z-c_k)^2) are smooth sigmoid-like functions
of z.  Each V_j is approximated (offline least squares) by a linear
combination of M=2 shifted erfs of common width plus a constant:
    V_j(z) ~= R[j,0] erf((z-g_0)/s) + R[j,1] erf((z-g_1)/s) + R[j,2]
so the device only computes the two basis sums D_k = sum_n erf((z'_n-g_k)/s)
per array and the host applies R in float64 plus the exact
eps/normalize/cumsum/mse tail.

Device layout: the 4 arrays per core (pred/target x 2 samples) are stacked
along the PARTITION axis -- array a owns partitions [32a, 32a+32), each
partition holding 2048 contiguous elements.  This makes the per-array
normalization constants per-partition vectors, which fuse directly into the
erf activation via its per-partition scale/bias operands: no separate
normalize pass.  The activation's accumulate output (accum_out) produces the
per-partition free-dim sums, so no vector-engine reduction chase either.

Per-core pipeline:
  - 6 chunked loads (free-chunks 512/512/1024 x {pred, target}, each
    spanning 2 samples = 64 partitions) over the three DMA-capable queues;
    DVE min/max reduces run per chunk as the data lands
  - cross-partition (per 32-partition group) min/max via a 5-round
    stream_shuffle XOR butterfly + tensor_tensor max, all on DVE
  - 5 small DVE ops turn (gmax, -gmin) into ACT scale/bias columns
  - 2 Erf activation passes [128, 2048] with per-partition scale/bias and
    accum_out -> red[128, 2]
  - red goes to DRAM; the host sums the 32-partition groups, applies R and
    computes the exact eps/normalize/cumsum/mse tail in float64
"""

import math

import numpy as np

B = 16
N = 65536
BINS = 64
SIGMA = 0.05
EPS = 1e-6
ALPHA = 0.5 / SIGMA**2  # 200.0
NCORES = 8
SPC = B // NCORES  # samples per core
NARR = 2 * SPC  # arrays per core: (pred, target) x samples
P = 128
GRP = P // NARR  # partitions per array
FT = N // GRP  # free dim per partition: 2048

# erf basis (designed offline; see module docstring)
M = 2
G0 = 0.397217865
G1 = 0.599198046
S_W = 0.136065682
CHUNKS = [0, 640, 1280, 2048]

_CACHE = {}


def _fit_R():
    """Least-squares fit of V_j(z) in the erf basis + constant (fp64)."""
    nz = 40001
    zg = np.linspace(0.0, 1.0, nz)
    c = np.linspace(0.0, 1.0, BINS)
    K = np.exp(-ALPHA * (zg[None, :] - c[:, None]) ** 2)
    Vt = np.cumsum(K, axis=0)  # [64, nz]
    erf_v = np.vectorize(math.erf)
    Fb = np.concatenate(
        [
            erf_v((zg[None, :] - np.array([[G0], [G1]])) / S_W),
            np.ones((1, nz)),
        ],
        axis=0,
    )  # [M+1, nz]
    w = np.ones(nz)
    w[zg < 0.02] = 3.0
    w[zg > 0.98] = 3.0
    Aw = Fb * w[None, :]
    Gm = Aw @ Fb.T
    Rhs = (Vt * w[None, :]) @ Fb.T
    return np.linalg.solve(Gm + 1e-10 * np.eye(M + 1), Rhs.T).T  # [64, M+1]


def _build_nc():
    import concourse.bass as bass  # noqa: F401
    import concourse.bacc as bacc
    import concourse.tile as tile
    from concourse import mybir
    from contextlib import ExitStack

    f32 = mybir.dt.float32
    f16 = mybir.dt.float16
    AX = mybir.AxisListType
    OP = mybir.AluOpType
    ACTF = mybir.ActivationFunctionType

    nc = bacc.Bacc()
    pred_d = nc.declare_dram_parameter("pred", [SPC, N], f32, isOutput=False)
    targ_d = nc.declare_dram_parameter("target", [SPC, N], f32, isOutput=False)
    out_d = nc.declare_dram_parameter("dsums", [P, M], f32, isOutput=True)

    with tile.TileContext(nc) as tc, ExitStack() as ctx:
        nat = ctx.enter_context(tc.tile_pool(name="nat", bufs=1))
        eo_pool = ctx.enter_context(tc.tile_pool(name="eo", bufs=2))
        small = ctx.enter_context(tc.tile_pool(name="small", bufs=1))

        # partition-stacked input: array a -> partitions [32a, 32a+32);
        # each dma spans one tensor (2 samples = 64 partitions) x free-chunk
        zn = nat.tile([P, FT], f32, name="zn")
        t2d = [
            pred_d.rearrange("s (p f) -> (s p) f", p=GRP),
            targ_d.rearrange("s (p f) -> (s p) f", p=GRP),
        ]
        # chunk 0 rides the two HWDGE queues (sync/scalar start ~0.65us
        # earlier than the gpsimd SWDGE path), so the DVE pipeline starts
        # as soon as possible; later chunks absorb the gpsimd latency
        qs = [nc.sync, nc.scalar, nc.gpsimd, nc.sync, nc.scalar, nc.gpsimd]
        qi = 0
        NCH = len(CHUNKS) - 1
        for ci in range(NCH):
            c0, c1 = CHUNKS[ci], CHUNKS[ci + 1]
            for t in range(2):
                qs[qi].dma_start(
                    out=zn[t * 2 * GRP : (t + 1) * 2 * GRP, c0:c1],
                    in_=t2d[t][:, c0:c1],
                )
                qi += 1

        # per-partition min/max, chunk-pipelined with the DMA
        mmp = small.tile([P, 2 * NCH], f32, tag="mmp")  # max cols | negmin cols
        for ci in range(NCH):
            c0, c1 = CHUNKS[ci], CHUNKS[ci + 1]
            nc.vector.tensor_reduce(
                out=mmp[:, ci : ci + 1], in_=zn[:, c0:c1], axis=AX.X, op=OP.max
            )
            nc.vector.tensor_reduce(
                out=mmp[:, NCH + ci : NCH + ci + 1], in_=zn[:, c0:c1], axis=AX.X,
                op=OP.min, negate=True,
            )
        mm = small.tile([P, 2], f32, tag="mm")  # (max, -min) per partition
        nc.vector.tensor_reduce(
            out=mm, in_=mmp.rearrange("p (t c) -> p t c", c=NCH), axis=AX.X,
            op=OP.max,
        )

        # 32-partition-group all-reduce max via DVE 32x32 block transpose:
        # broadcast each partition's (max, -min) along 32 free cols; the
        # in-place block transpose then puts the whole quadrant's values in
        # the free dim of every partition; one segmented reduce finishes
        bb = small.tile([P, 64], f32, tag="bb")
        nc.vector.tensor_scalar(
            bb[:, 0:32], zn[:, 0:32], 0.0, mm[:, 0:1], OP.mult, OP.add
        )
        nc.vector.tensor_scalar(
            bb[:, 32:64], zn[:, 0:32], 0.0, mm[:, 1:2], OP.mult, OP.add
        )
        bt = small.tile([P, 64], f32, tag="bt")
        nc.vector.transpose(bt, bb)
        cur = small.tile([P, 2], f32, tag="gm")
        nc.vector.tensor_reduce(
            out=cur, in_=bt.rearrange("p (c f) -> p c f", f=32), axis=AX.X,
            op=OP.max,
        )
        # cur[p] = (gmax, -gmin) of partition p's array, on every partition

        # scale/bias columns: nb = [inv/S | -gmin*inv/S - G0/S | ... - G1/S],
        # inv = 1/(gmax-gmin); bias_k fused as (-gmin)*(inv/S) + (-Gk/S)
        nb = small.tile([P, 1 + M], f32, tag="nb")
        rngs = small.tile([P, 1], f32, tag="rngs")
        nc.vector.tensor_scalar(
            rngs, cur[:, 0:1], cur[:, 1:2], float(S_W), OP.add, OP.mult
        )
        nc.vector.reciprocal(nb[:, 0:1], rngs)
        nc.vector.tensor_scalar(
            nb[:, 1:2], cur[:, 1:2], nb[:, 0:1], float(-G0 / S_W), OP.mult, OP.add
        )
        nc.vector.tensor_scalar(
            nb[:, 2:3], cur[:, 1:2], nb[:, 0:1], float(-G1 / S_W), OP.mult, OP.add
        )

        # erf spine: arg = z*(inv/S) + bias_k = (z' - g_k)/S; accum_out gives
        # the per-partition free-dim sums directly
        red = small.tile([P, M], f32, tag="red")
        for k in range(M):
            eo = eo_pool.tile([P, FT], f16, tag="eo")
            nc.scalar.activation(
                out=eo,
                in_=zn,
                func=ACTF.Erf,
                bias=nb[:, 1 + k : 2 + k],
                scale=nb[:, 0:1],
                accum_out=red[:, k : k + 1],
            )

        # issue from the scalar queue: the accum-read that produces red runs
        # on Scalar, so no cross-engine semaphore hop before the out DMA
        nc.scalar.dma_start(out=out_d[:, :], in_=red)

    nc.compile()
    return nc


def kernel(pred: np.ndarray, target: np.ndarray) -> np.ndarray:
    from concourse.bass_utils import run_bass_kernel_spmd

    if "nc" not in _CACHE:
        _CACHE["nc"] = _build_nc()
        _CACHE["R"] = _fit_R()
    nc = _CACHE["nc"]
    R = _CACHE["R"]

    pred = np.ascontiguousarray(np.asarray(pred, np.float32).reshape(B, N))
    target = np.ascontiguousarray(np.asarray(target, np.float32).reshape(B, N))
    in_maps = [
        {
            "pred": pred[i * SPC : (i + 1) * SPC],
            "target": target[i * SPC : (i + 1) * SPC],
        }
        for i in range(NCORES)
    ]
    res = run_bass_kernel_spmd(nc, in_maps, list(range(NCORES)))

    # device dsums [128, M]; array a = partition rows [32a, 32a+32) with
    # a in (pred_s0, pred_s1, targ_s0, targ_s1); host sums the groups
    Dp = np.zeros((B, M + 1))
    Dt = np.zeros((B, M + 1))
    Dp[:, M] = N
    Dt[:, M] = N
    for core in range(NCORES):
        raw = np.asarray(res.results[core]["dsums"], np.float64)  # [128, M]
        for s in range(SPC):
            b = core * SPC + s
            Dp[b, :M] = raw[s * GRP : (s + 1) * GRP, :].sum(axis=0)
            Dt[b, :M] = raw[(SPC + s) * GRP : (SPC + s + 1) * GRP, :].sum(axis=0)

    Sx = Dp @ R.T  # [B, 64] cumulative kernel sums
    Sy = Dt @ R.T
    js = np.arange(1, BINS + 1, dtype=np.float64)
    cdf_x = (Sx + js[None, :] * EPS) / (Sx[:, -1:] + BINS * EPS)
    cdf_y = (Sy + js[None, :] * EPS) / (Sy[:, -1:] + BINS * EPS)
    return np.float32(np.mean((cdf_x - cdf_y) ** 2))
